# revision 41
# baseline (speedup 1.0000x reference)
"""Trainium2 Bass kernel for KVAdapterInjector (Qwen3-style GQA attention with
LoRA-adapted virtual KV prefix).

Sharding: tensor-parallel over heads across 8 cores. Core m gets KV head m and
Q heads 4m..4m+3. Wq/Wk/Wv sharded on output dim, Wo on input dim; partial
outputs (bf16) summed on host.

v4 design notes (CoreSim cost-model driven):
- Heavy matmuls bf16 (1.0 cyc/row), except 6 of 32 contraction d-tiles of the
  Q projection (d26..31) which run as 3 fp8e4 DoubleRow matmuls per pass
  (0.5 cyc/row on the moving dim with 2 k-tiles per instruction). Operands
  are host-prescaled (Wq x8, hs /8) to dodge the e4m3 subnormal range; fp32
  PSUM accumulation. Measured end-to-end rel err 1.85e-2 (budget 2e-2);
  error is deterministic run-to-run.
- Virtual KV (64 tokens) merged into key block 0: kT is laid out with virt
  keys at cols 0:64 and real key t at col 64+t, so block b = cols
  [128b, 128b+128). vnat rows shifted by 64 via 2 SBUF->SBUF partition-shift
  DMAs per chunk from a staging tile. Diagonal blocks per chunk: j=0..4 with
  query-trimmed widths 512,448,320,192,64 (j=4 uses only 64 key rows so it
  never reads unwritten kT/vnat regions).
- Startup: PE warmup matmuls from t~0.9us anchor the p-state ramp (any PE gap
  < ~3.5us never resets it) and fill the DMA-bound startup holes; first-pass
  loads split into small pieces across SP/Act/Pool DMA queues; pass-q0
  matmuls emitted in data-availability order.
- Phase pipeline: proj passes of chunk c+1 and outproj units of chunk c-1 are
  rationed-interleaved between attention blocks of chunk c (QK emitted 2-3
  blocks ahead of PV so Act exp latency is hidden). The final outproj
  pre-accumulates h0..h2 for units {0..4,31} during the last den-chain wait;
  unit 31 closes with a single matmul to shorten the drain+store tail.
- Stores: flat contiguous DRAM layout outp(16,2,2,128,1024) -> each store is
  a [128,1024] contiguous write costing the 500ns descriptor-gen floor.
- Engine budget: PE ~334us, Act ~150 (exps+norm+half drains), Pool ~150
  (blocksums+broadcast), DVE ~130 (rope, masks, normalize, half drains).
"""
import sys

sys.path.insert(0, "/opt/trn_rl_repo")

from collections import deque

import numpy as np
import ml_dtypes

import concourse.bass as bass
import concourse.mybir as mybir
import concourse.tile as tile
from concourse import bacc
from concourse import bass_isa
from concourse.bass_utils import run_bass_kernel_spmd

F32 = mybir.dt.float32
F8 = mybir.dt.float8e4
BF16 = mybir.dt.bfloat16
AX = mybir.AxisListType
ALU = mybir.AluOpType
ACTF = mybir.ActivationFunctionType
RED = bass_isa.ReduceOp

T = 2048
D = 4096
HD = 128
NQH = 4          # q heads per core
R = 64           # virtual tokens
RANK = 16
EPS = 1e-6
SCALING = HD ** -0.5
NTC = 4          # T chunks of 512
TC = 512
ND = D // 128    # 32 contraction tiles
NKB = T // 128   # 16 key blocks (real)

# pass index order in wpp: q0, k, v, q1, q2, q3
PASS_Q = [0, 3, 4, 5]   # wpp index of q-head passes 0..3
PASS_K = 1
PASS_V = 2


def build_nc():
    nc = bacc.Bacc(None, target_bir_lowering=False, debug=False)

    # ---- DRAM I/O ----
    wpp = nc.dram_tensor("wpp", (6, 128, ND * 128), BF16, kind="ExternalInput")
    hsp = nc.dram_tensor("hsp", (NTC * 4, 128, 8 * TC), BF16, kind="ExternalInput")
    wo = nc.dram_tensor("wo", (NQH * HD, D), BF16, kind="ExternalInput")
    cwq = nc.dram_tensor("cwq", (HD, T), BF16, kind="ExternalInput")
    swq = nc.dram_tensor("swq", (HD, T), BF16, kind="ExternalInput")
    cwk = nc.dram_tensor("cwk", (HD, T), BF16, kind="ExternalInput")
    swk = nc.dram_tensor("swk", (HD, T), BF16, kind="ExternalInput")
    masktri = nc.dram_tensor("masktri", (128, 128), F32, kind="ExternalInput")
    mask64 = nc.dram_tensor("mask64", (128, 64), F32, kind="ExternalInput")
    kvirt = nc.dram_tensor("kvirt", (HD, R), BF16, kind="ExternalInput")
    vvirt = nc.dram_tensor("vvirt", (R, HD), BF16, kind="ExternalInput")
    wq8 = nc.dram_tensor("wq8", (4, 128, 3, 2, 128), F8,
                         kind="ExternalInput")
    hsp8 = nc.dram_tensor("hsp8", (NTC, 128, 3, 2, TC), F8,
                          kind="ExternalInput")
    outp = nc.dram_tensor("outp", (16, 2, 2, 128, 1024), BF16,
                          kind="ExternalOutput")

    from contextlib import ExitStack
    with tile.TileContext(nc) as tc, ExitStack() as est:
        cp = est.enter_context(tc.tile_pool(name="consts", bufs=1))
        pp = est.enter_context(tc.tile_pool(name="persist", bufs=1))

        # pin the Act table serving square+ln+exp
        from concourse.hw_specs import get_activation_tables
        _tables = list(get_activation_tables(nc.m.arch).keys())
        _atl = mybir.InstLoadActFuncSet(
            name=nc.get_next_instruction_name(), ins=[], outs=[],
            act_func_set_id=_tables.index("natural_log_exp_and_others"))
        _atl.engine = mybir.EngineType.Activation
        nc.scalar.add_instruction(_atl)

        # ---- small consts ----
        epsc = cp.tile([128, 1], F32)
        nc.vector.memset(epsc[:], EPS)
        zeroc = cp.tile([128, 1], F32)
        nc.vector.memset(zeroc[:], 0.0)
        warmx = cp.tile([128, 256], BF16)
        nc.vector.memset(warmx[:], 0.125)
        mask_s = cp.tile([128, 128], F32)
        mask64_s = cp.tile([128, 64], F32)

        # ---- persistent activations ----
        qT = [pp.tile([HD, T], BF16, tag=f"qT{h}", name=f"qT{h}") for h in range(NQH)]
        oT = qT
        kT = pp.tile([HD, R + T], BF16)            # col g: g<64 virt, g=64+t real
        vnat = pp.tile([128, NKB + 1, 128], BF16)  # row g%128 of block g//128

        # ---- rope/norm consts ----
        cwq_s = cp.tile([HD, T], BF16)
        swq_s = cp.tile([HD, T], BF16)
        cwk_s = cp.tile([HD, T], BF16)
        swk_s = cp.tile([HD, T], BF16)

        # ---- weights in SBUF (separate tiles per pass for dep granularity) --
        wq0a = cp.tile([128, 2, 128], BF16)    # pass q0 d0,d1
        wq0b = cp.tile([128, 30, 128], BF16)   # pass q0 d2..31
        wps = [None] * 6                       # indexed by wpp pass id
        for pid in (PASS_K, PASS_V, PASS_Q[1], PASS_Q[2], PASS_Q[3]):
            wps[pid] = cp.tile([128, ND, 128], BF16, tag=f"wp{pid}",
                               name=f"wp{pid}")
        wo_s = cp.tile([128, NQH, D], BF16)
        wq8_s = cp.tile([128, 4, 3, 2, 128], F8)

        def wap(pid, d):
            if pid == 0:
                return wq0a[:, d, :] if d < 2 else wq0b[:, d - 2, :]
            return wps[pid][:, d, :]

        pm = lambda ap: ap.rearrange("(n p) c -> p n c", p=128)

        with tc.tile_pool(name="proj_ps", bufs=2, space="PSUM") as prps, \
             tc.tile_pool(name="mm_ps", bufs=4, space="PSUM") as mmps, \
             tc.tile_pool(name="b2k_ps", bufs=2, space="PSUM") as b2ps, \
             tc.tile_pool(name="hs_sb", bufs=1) as hsb, \
             tc.tile_pool(name="nrm_sb", bufs=2) as nsb, \
             tc.tile_pool(name="pe_sb", bufs=2) as peb, \
             tc.tile_pool(name="at_sb", bufs=2) as asb, \
             tc.tile_pool(name="vs_sb", bufs=2) as vsb, \
             tc.tile_pool(name="ob_sb", bufs=2) as obb:

            # ---- PE warmup: keep pe ramp alive until first real matmul ----
            for _ in range(14):
                wmp = mmps.tile([128, TC], F32, tag="mm", name="warm")
                nc.tensor.matmul(wmp[:, 0:256], warmx[:, 0:128], warmx[:],
                                 start=True, stop=True)

            # ---- hs tiles: 4x [128,8,TC] per chunk; chunk 0 loaded via
            # sliced DMAs across 3 queues for early availability ----
            hs_tiles = {}
            hs8_tiles = {}

            def new_hs(c):
                hs_tiles[c] = [hsb.tile([128, 8, TC], BF16, tag=f"hs{i}",
                                        name=f"hs{c}_{i}") for i in range(4)]
                hs8_tiles[c] = hsb.tile([128, 3, 2, TC], F8, tag="hs8",
                                        name=f"hs8_{c}")
                return hs_tiles[c]

            def hs_ap(c, d):
                return hs_tiles[c][d // 8][:, d % 8, :]

            def load_hs(c):
                new_hs(c)
                for i in range(4):
                    nc.sync.dma_start(hs_tiles[c][i][:], hsp[4 * c + i])
                nc.sync.dma_start(hs8_tiles[c][:], hsp8[c])

            # ---- startup loads ----
            # SP queue
            nc.sync.dma_start(wq0a[:], wpp[0][:, 0:256].rearrange(
                "p (d x) -> p d x", d=2))
            nc.sync.dma_start(wq0b[:], wpp[0][:, 256:4096].rearrange(
                "p (d x) -> p d x", d=30))
            nc.sync.dma_start(wps[PASS_K][:], wpp[PASS_K].rearrange(
                "p (d x) -> p d x", d=ND))
            nc.sync.dma_start(wps[PASS_V][:], wpp[PASS_V].rearrange(
                "p (d x) -> p d x", d=ND))
            nc.sync.dma_start(wq8_s[:], wq8.rearrange("q p i j m -> p q i j m"))
            nc.sync.dma_start(mask_s[:], masktri[:])
            nc.sync.dma_start(mask64_s[:], mask64[:])
            nc.sync.dma_start(kT[:, 0:R], kvirt[:])
            nc.sync.dma_start(vnat[0:R, 0, :], vvirt[:])
            for pid in (PASS_Q[1], PASS_Q[2], PASS_Q[3]):
                nc.sync.dma_start(wps[pid][:], wpp[pid].rearrange(
                    "p (d x) -> p d x", d=ND))
            # Act queue (startup only)
            hs0 = new_hs(0)
            nc.scalar.dma_start(hs0[0][:, 0, :], hsp[0][:, 0:TC])
            nc.scalar.dma_start(hs0[0][:, 1, :], hsp[0][:, TC:2 * TC])
            nc.scalar.dma_start(hs0[0][:, 2:8, :], hsp[0][:, 2 * TC:8 * TC]
                                .rearrange("p (d x) -> p d x", d=6))
            nc.scalar.dma_start(hs0[3][:], hsp[3].rearrange(
                "p (d x) -> p d x", d=8))
            nc.scalar.dma_start(cwq_s[:], cwq[:])
            nc.scalar.dma_start(swq_s[:], swq[:])
            nc.scalar.dma_start(cwk_s[:], cwk[:])
            nc.scalar.dma_start(swk_s[:], swk[:])
            # Pool queue (SWDGE, startup only)
            nc.gpsimd.dma_start(hs0[1][:, 0:4, :], hsp[1][:, 0:4 * TC]
                                .rearrange("p (d x) -> p d x", d=4))
            nc.gpsimd.dma_start(hs0[1][:, 4:8, :], hsp[1][:, 4 * TC:8 * TC]
                                .rearrange("p (d x) -> p d x", d=4))
            nc.gpsimd.dma_start(hs0[2][:], hsp[2].rearrange(
                "p (d x) -> p d x", d=8))
            nc.gpsimd.dma_start(hs8_tiles[0][:], hsp8[0])

            # ================= helpers =================

            def emit_norm(c, pid, qh, pacc):
                """rms-norm + rope of one projection pass (PSUM pacc)."""
                ts = slice(c * TC, (c + 1) * TC)
                isq = qh is not None
                cw = cwq_s if isq else cwk_s
                sw = swq_s if isq else swk_s
                dst = qT[qh][:, ts] if isq else kT[:, R + c * TC: R + (c + 1) * TC]
                sq = nsb.tile([HD, TC], BF16, tag="sq")
                nc.scalar.activation(sq[:], pacc[:], ACTF.Square, bias=zeroc[:])
                ssum = nsb.tile([HD, TC], BF16, tag="ssum")
                nc.gpsimd.partition_all_reduce(ssum[:], sq[:], channels=128,
                                               reduce_op=RED.add)
                lns = nsb.tile([HD, TC], F32, tag="lns")
                nc.scalar.activation(lns[:], ssum[:], ACTF.Ln,
                                     scale=1.0 / HD, bias=epsc[:])
                rinv = nsb.tile([HD, TC], BF16, tag="rinv")
                nc.scalar.activation(rinv[:], lns[:], ACTF.Exp, scale=-0.5,
                                     bias=zeroc[:])
                xn = nsb.tile([HD, TC], BF16, tag="xn")
                nc.vector.tensor_mul(xn[:], pacc[:], rinv[:])
                t1 = nsb.tile([HD, TC], BF16, tag="t1")
                nc.vector.tensor_mul(t1[:], xn[:], cw[:, ts])
                rot = nsb.tile([HD, TC], BF16, tag="rot")
                nc.scalar.dma_start(rot[0:64, :], xn[64:128, :])
                nc.scalar.dma_start(rot[64:128, :], xn[0:64, :])
                nc.vector.tensor_mul(rot[:], rot[:], sw[:, ts])
                nc.vector.tensor_add(dst, t1[:], rot[:])

            def gen_qk_pass(c, pid, qh, dorder=None):
                """Generator: one projection pass in quarters. q passes do
                d0..27 in bf16 plus d28..31 as 2 fp8 DoubleRow matmuls
                (operands pre-scaled x8 / /8 on the host)."""
                pacc = prps.tile([128, TC], F32, tag="pacc",
                                 name=f"pacc{c}_{pid}")
                isq = qh is not None
                nbf = 26 if isq else ND
                ds = dorder if dorder is not None else list(range(nbf))
                qsz = (nbf + 3) // 4
                for qtr in range(4):
                    for i in range(qtr * qsz, min((qtr + 1) * qsz, nbf)):
                        d = ds[i]
                        nc.tensor.matmul(pacc[:], wap(pid, d), hs_ap(c, d),
                                         start=(i == 0),
                                         stop=(not isq and i == ND - 1))
                    if qtr < 3:
                        yield
                if isq:
                    for i in range(3):
                        nc.tensor.matmul(
                            pacc[:], wq8_s[:, qh, i, :, :],
                            hs8_tiles[c][:, i, :, :],
                            start=False, stop=(i == 2),
                            perf_mode=mybir.MatmulPerfMode.DoubleRow)
                emit_norm(c, pid, qh, pacc)
                yield

            def gen_v_pass(c):
                """Generator: V natural-layout pass + staging drain + shift."""
                vacc = b2ps.tile([128, 4, 128], F32, tag="b2k",
                                 name=f"vacc{c}")
                for tt in range(4):
                    for d in range(ND):
                        nc.tensor.matmul(vacc[:, tt, :],
                                         hs_ap(c, d)[:, tt * 128:(tt + 1) * 128],
                                         wap(PASS_V, d),
                                         start=(d == 0), stop=(d == ND - 1))
                    if tt < 3:
                        yield
                vst = vsb.tile([128, 4, 128], BF16, tag="vst", name=f"vst{c}")
                nc.vector.tensor_copy(vst[:], vacc[:])
                # partition-shift by +64 into vnat blocks
                nc.sync.dma_start(vnat[64:128, 4 * c:4 * c + 4, :],
                                  vst[0:64, :, :])
                nc.sync.dma_start(vnat[0:64, 4 * c + 1:4 * c + 5, :],
                                  vst[64:128, :, :])
                yield

            def gen_proj(c, first=False):
                """All projection passes of chunk c (as one generator)."""
                if c > 0:
                    load_hs(c)
                order = [(PASS_Q[0], 0), (PASS_K, None), (PASS_V, "v"),
                         (PASS_Q[1], 1), (PASS_Q[2], 2), (PASS_Q[3], 3)]
                if not first:
                    # k-pass last: minimizes kT write/read window overlap
                    order = [(PASS_Q[0], 0), (PASS_V, "v"), (PASS_Q[1], 1),
                             (PASS_Q[2], 2), (PASS_Q[3], 3), (PASS_K, None)]
                for pid, qh in order:
                    if qh == "v":
                        yield from gen_v_pass(c)
                    else:
                        yield from gen_qk_pass(c, pid, qh)

            def gen_outproj(cc, final=False, pool=None):
                """Generator: output projection of chunk cc, 32 units.

                final: the first NE units' h0..h2 matmuls are emitted up
                front (they don't need the last head's normalize), so the PE
                keeps running while the last den chain completes."""
                if final:
                    EARLY = (0, 1, 2, 3, 4, 31)
                else:
                    # only the phase-3 filler (no proj contends for prps)
                    EARLY = (0, 1) if cc == NTC - 2 else ()
                early = {}
                for n_, u in enumerate(EARLY):
                    tt = 4 * cc + u // 8
                    half = (u // 4) % 2
                    j2 = 4 * half + u % 4
                    if final:
                        pl, tg = (mmps, "mm") if n_ < 4 else (prps, "pacc")
                    else:
                        pl, tg = prps, "pacc"
                    po2 = pl.tile([128, TC], F32, tag=tg,
                                  name=f"fop{cc}_{u}")
                    for h2 in range(NQH - 1):
                        nc.tensor.matmul(
                            po2[:], oT[h2][:, tt * 128:(tt + 1) * 128],
                            wo_s[:, h2, j2 * TC:(j2 + 1) * TC],
                            start=(h2 == 0), stop=False)
                    early[u] = po2
                for u in range(32):
                    tt = 4 * cc + u // 8
                    half = (u // 4) % 2
                    jj = u % 4
                    j2 = 4 * half + jj
                    if jj == 0:
                        ob = obb.tile([128, 4, TC], BF16, tag="ob",
                                      name=f"ob{cc}_{u}")
                        gen_outproj.ob = ob
                    ob = gen_outproj.ob
                    if u in early:
                        po2 = early[u]
                        nc.tensor.matmul(
                            po2[:], oT[NQH - 1][:, tt * 128:(tt + 1) * 128],
                            wo_s[:, NQH - 1, j2 * TC:(j2 + 1) * TC],
                            start=False, stop=True)
                    else:
                        po2 = (pool or mmps).tile(
                            [128, TC], F32,
                            tag="pacc" if pool is not None else "mm")
                        for h2 in range(NQH):
                            nc.tensor.matmul(
                                po2[:], oT[h2][:, tt * 128:(tt + 1) * 128],
                                wo_s[:, h2, j2 * TC:(j2 + 1) * TC],
                                start=(h2 == 0), stop=(h2 == NQH - 1))
                    use_dve = (u % 2 == 0) if final else (jj != 3)
                    if use_dve:
                        nc.vector.tensor_copy(ob[:, jj, :], po2[:])
                    else:
                        nc.scalar.copy(ob[:, jj, :], po2[:])
                    if jj % 2 == 1:
                        sub = (jj - 1) // 2
                        nc.sync.dma_start(
                            outp[tt, half, sub].rearrange("p c -> (p c)"),
                            ob[:, jj - 1:jj + 1, :])
                    yield

            # ---- attention ----
            DIAG_W = [TC, 448, 320, 192, 64]
            DIAG_Q0 = [0, 64, 192, 320, 448]

            def emit_attention(c, pump, fill_rate=1.0, skip_slots=0, depth=2):
                ts = slice(c * TC, (c + 1) * TC)
                B = 4 * c + 5
                credit = [0.0]
                slot = [0]

                def rationed_pump():
                    slot[0] += 1
                    if slot[0] <= skip_slots:
                        return
                    credit[0] += fill_rate
                    n = int(credit[0])
                    if n:
                        credit[0] -= n
                        pump(n)

                for h in range(NQH):
                    nparity = 2 if c > 0 else 1
                    denp = [asb.tile([1, TC], F32, tag=f"den{i}", bufs=2,
                                     name=f"den{i}")
                            for i in range(nparity)]
                    po = b2ps.tile([128, TC], F32, tag="b2k")
                    blk_i = [0]

                    def blocksum(pe_ap, q0):
                        i = blk_i[0]
                        blk_i[0] += 1
                        den_ = denp[i % nparity]
                        rows = pe_ap.shape[0]
                        W_ = pe_ap.shape[-1]
                        ps_ = peb.tile([128, TC], BF16, tag="ps", bufs=3)
                        nc.gpsimd.partition_all_reduce(
                            ps_[0:rows, 0:W_], pe_ap, channels=rows,
                            reduce_op=RED.add)
                        if i < nparity:
                            nc.gpsimd.tensor_copy(den_[:], ps_[0:1, 0:W_])
                        else:
                            nc.gpsimd.tensor_add(den_[:, q0:TC],
                                                 den_[:, q0:TC],
                                                 ps_[0:1, 0:W_])

                    def qk_block(i):
                        b = i
                        if i < 4 * c:
                            W, q0, krows, mk = TC, 0, 128, None
                        else:
                            j = i - 4 * c
                            W = DIAG_W[j]
                            q0 = DIAG_Q0[j]
                            krows = 64 if j == 4 else 128
                            mk = j
                        st = mmps.tile([128, TC], F32, tag="mm")
                        nc.tensor.matmul(
                            st[0:krows, 0:W], kT[:, 128 * b:128 * b + krows],
                            qT[h][:, c * TC + q0:(c + 1) * TC],
                            start=True, stop=True)
                        if mk is not None:
                            j = mk
                            if j == 0:
                                nc.vector.tensor_add(st[:, 0:64], st[:, 0:64],
                                                     mask64_s[:])
                            elif j < 4:
                                nc.vector.tensor_add(st[:, 0:128], st[:, 0:128],
                                                     mask_s[:])
                            else:
                                nc.vector.tensor_add(st[0:64, 0:64],
                                                     st[0:64, 0:64],
                                                     mask_s[0:64, 0:64])
                        pe = peb.tile([128, TC], BF16, tag="pe", bufs=5)
                        nc.scalar.activation(pe[0:krows, 0:W], st[0:krows, 0:W],
                                             ACTF.Exp, scale=SCALING,
                                             bias=zeroc[0:krows, :])
                        blocksum(pe[0:krows, 0:W], q0)
                        return pe, W, q0, krows, b

                    def pv_block(blk, i):
                        pe, W, q0, krows, b = blk
                        nc.tensor.matmul(po[:, q0:TC],
                                         vnat[0:krows, b, :],
                                         pe[0:krows, 0:W],
                                         start=(i == 0), stop=(i == B - 1))

                    # software pipeline: QK runs `depth` blocks ahead of PV
                    # so the Act exp latency is always hidden
                    pipe = [qk_block(i) for i in range(min(depth, B))]
                    for i in range(B):
                        if i + depth < B:
                            pipe.append(qk_block(i + depth))
                        rationed_pump()
                        pv_block(pipe[i], i)
                    # normalize
                    if nparity == 2:
                        dsum = asb.tile([1, TC], F32, tag="dsum", bufs=1)
                        nc.vector.tensor_add(dsum[:], denp[0][:], denp[1][:])
                    else:
                        dsum = denp[0]
                    rc = asb.tile([1, TC], BF16, tag="rc")
                    with nc.allow_low_precision(reason="softmax denom bf16"):
                        nc.vector.reciprocal(rc[:], dsum[:])
                    rb = asb.tile([128, TC], BF16, tag="rb", bufs=1)
                    nc.gpsimd.partition_broadcast(rb[:], rc[:], channels=128)
                    nc.vector.tensor_mul(oT[h][:, ts], po[:], rb[:])
                    rationed_pump()

            # ================= schedule =================
            fillers = deque()

            def pump(n=1):
                done = 0
                while done < n and fillers:
                    try:
                        next(fillers[0])
                        done += 1
                    except StopIteration:
                        fillers.popleft()

            # startup: chunk-0 projections emitted directly, q0 in
            # data-availability order
            d_order0 = [0, 1, 8, 9, 10, 11, 12, 13, 14, 15, 2, 3, 4, 5, 6, 7,
                        16, 17, 18, 19, 20, 21, 22, 23, 24, 25]
            for _ in gen_qk_pass(0, PASS_Q[0], 0, dorder=d_order0):
                pass
            for _ in gen_qk_pass(0, PASS_K, None):
                pass
            for _ in gen_v_pass(0):
                pass
            for _ in gen_qk_pass(0, PASS_Q[1], 1):
                pass
            for _ in gen_qk_pass(0, PASS_Q[2], 2):
                pass
            for _ in gen_qk_pass(0, PASS_Q[3], 3):
                pass
            nc.sync.dma_start(wo_s[:], pm(wo[:, :]))

            # phase 0: attention(c0) + proj(c1)
            fillers.append(gen_proj(1))
            emit_attention(0, pump, fill_rate=0.9)
            pump(99)

            # phases 1..3 (fill_rate ~= filler_units / pump_slots, slightly
            # under so leftovers drain at the phase tail; skip the first few
            # slots so the previous chunk's den chains complete before the
            # first outproj filler can block the PE queue)
            RATES = {1: 1.25, 2: 0.76, 3: 0.36}
            for c in range(1, NTC):
                fillers.append(gen_outproj(c - 1, final=False,
                                           pool=prps if c == 3 else None))
                if c + 1 < NTC:
                    fillers.append(gen_proj(c + 1))
                emit_attention(c, pump, fill_rate=RATES[c], skip_slots=3,
                               depth=3 if c == 3 else 2)
                pump(99)

            # final outproj
            for _ in gen_outproj(NTC - 1, final=True):
                pass

    nc.compile()
    return nc


_NC_CACHE = {}


def _get_nc():
    if "nc" not in _NC_CACHE:
        _NC_CACHE["nc"] = build_nc()
    return _NC_CACHE["nc"]


def _bf(x):
    return np.ascontiguousarray(x.astype(ml_dtypes.bfloat16))


def kernel(**inputs) -> np.ndarray:
    f = lambda k: np.asarray(inputs[k], np.float32)
    hs = f("hidden_states")[0]            # (T, D)
    vk = f("virtual_keys")[0]             # (HKV, R, HD)
    vv = f("virtual_values")[0]
    Wq, Wk, Wv, Wo = f("Wq"), f("Wk"), f("Wv"), f("Wo")
    qnw, knw = f("q_norm_w"), f("k_norm_w")
    lkA, lkB = f("lora_k_A"), f("lora_k_B")
    lvA, lvB = f("lora_v_A"), f("lora_v_B")
    sk = np.float32(np.asarray(inputs["scale_k"]))
    sv = np.float32(np.asarray(inputs["scale_v"]))
    cos, sin = f("cos"), f("sin")         # (T, HD)

    # packed tiles: hsp[c*4+i][p][d8*TC+t] = hs[c*TC+t, (8i+d8)*128+p]
    hsT32 = hs.T.reshape(ND, 128, NTC, TC)          # [dtile, p, c, t]
    hsp = _bf(hsT32.transpose(2, 0, 1, 3)           # [c, dtile, p, t]
              .reshape(NTC, 4, 8, 128, TC)
              .transpose(0, 1, 3, 2, 4)
              .reshape(NTC * 4, 128, 8 * TC))

    def cw_sw(w):
        cw = (cos.T * w[:, None]).astype(np.float32)
        sw = np.empty((HD, T), np.float32)
        sw[0:64] = -w[64:128, None] * sin.T[0:64]
        sw[64:128] = w[0:64, None] * sin.T[64:128]
        return _bf(cw), _bf(sw)
    hsp8 = np.ascontiguousarray(
        (hs.T.reshape(ND, 128, NTC, TC)[26:32] / 8.0)
        .reshape(3, 2, 128, NTC, TC).transpose(3, 2, 0, 1, 4)
        .astype(mybir.dt.np(mybir.dt.float8e4)))
    cwqh, swqh = cw_sw(qnw)
    cwkh, swkh = cw_sw(knw)
    # host-side LoRA adaptation of the virtual KV (tiny)
    vk_a = vk + sk * (vk @ lkA @ lkB)      # (HKV, R, HD)
    vv_a = vv + sv * (vv @ lvA @ lvB)
    # mask tiles: masktri[r,m] = 0 if r<=m ; mask64[r,m] = 0 if r<=m+64
    idx = np.arange(128)
    masktri = np.where(idx[:, None] <= idx[None, :], 0.0, -1e30).astype(np.float32)
    mask64 = np.where(idx[:, None] <= idx[None, :64] + 64, 0.0,
                      -1e30).astype(np.float32)

    # wpp pass order: q0, k, v, q1, q2, q3
    f8np = mybir.dt.np(mybir.dt.float8e4)

    def wq8_m(m):
        A = (Wq[:, 512 * m:512 * (m + 1)] * 8.0).reshape(ND, 128, 4, 128)
        return np.ascontiguousarray(
            A[26:32].reshape(3, 2, 128, 4, 128).transpose(3, 2, 0, 1, 4)
            .astype(f8np))

    def wpp_m(m):
        cols = [Wq[:, 512 * m:512 * m + 128],
                Wk[:, 128 * m:128 * (m + 1)],
                Wv[:, 128 * m:128 * (m + 1)],
                Wq[:, 512 * m + 128:512 * m + 256],
                Wq[:, 512 * m + 256:512 * m + 384],
                Wq[:, 512 * m + 384:512 * m + 512]]
        blocks = [c.reshape(ND, 128, 128).transpose(1, 0, 2).reshape(128, ND * 128)
                  for c in cols]
        return _bf(np.stack(blocks, axis=0))

    in_maps = []
    for m in range(8):
        in_maps.append({
            "hsp": hsp,
            "wpp": wpp_m(m),
            "wo": _bf(Wo[512 * m:512 * (m + 1), :]),
            "cwq": cwqh, "swq": swqh, "cwk": cwkh, "swk": swkh,
            "masktri": masktri,
            "mask64": mask64,
            "kvirt": _bf(vk_a[m].T),
            "vvirt": _bf(vv_a[m]),
            "wq8": wq8_m(m),
            "hsp8": hsp8,
        })

    nc = _get_nc()
    res = run_bass_kernel_spmd(nc, in_maps, core_ids=list(range(8)))
    acc = None
    for m in range(8):
        o = np.asarray(res.results[m]["outp"]).astype(np.float32)
        o = o.reshape(16, 2, 2, 128, 1024).transpose(0, 3, 1, 2, 4).reshape(T, D)
        acc = o if acc is None else acc + o
    return acc[None]  # (1, T, D)
